# revision 2
# baseline (speedup 1.0000x reference)
"""BiDirectional LSTM (B=32, T=512, D=H=512, hard_sigmoid gates, output=fwd+bwd sum)
on 8 Trainium2 NeuronCores — v6.

Sharding: core c in 0..7 -> direction d = c//4 (0=fwd, 1=bwd), batch shard s = c%4.
Backward direction realized in data (time-reversed inputs, Theano go_backwards).

v5 (from v4 trace): the tail is DVE-throughput-bound — after the 3rd mm-group's
~370ns completion latency, ~7 serial DVE ops gate the next step. Changes:
  - gate order f, c~, i, o: the f-chain (zf, sf, t1) retires mid-stream; the tail
    window holds only zi, si, t2, ca, zo, so, h.
  - bf16 gate tiles (z/s/gt/t2/h): DVE 2x/4x perf modes (c accumulator stays fp32).
  - c_st lives in PSUM: tanh(c) reads ScalarE's faster PSUM port.
  - phase-1 GEMM fully folded into the stream via a work queue (2 items/step in
    block 0, then 1/step); prologue is just block 0.
  - explicit DVE/ACT/PE ordering via add_dep_helper.
"""

import numpy as np
import ml_dtypes

B, T, D, H = 32, 512, 512, 512
NCORES = 8
BC = B // 4          # 8 samples per core
KT = D // 128        # 4 k-tiles
MT = (4 * H) // 128  # 16 m-tiles; gate col order [i|f|o|c]
GM = {"i": 0, "f": 4, "o": 8, "c": 12}  # m-chunk base per gate


def build(nc, Tn=T):
    import concourse.mybir as mybir
    from concourse.tile import TileContext
    from concourse.alu_op_type import AluOpType
    from concourse.tile_rust import add_dep_helper

    f32 = mybir.dt.float32
    bf16 = mybir.dt.bfloat16
    AF = mybir.ActivationFunctionType
    SDT = bf16  # gate tiles in bf16 (DVE 2x/4x modes); c accumulator stays fp32
    NT = Tn * BC       # GEMM moving free size
    TBLK = 64          # steps per block; one block = one 512-wide GEMM n-chunk
    NCK = TBLK * BC    # 512
    assert Tn % TBLK == 0
    NBLK = Tn // TBLK

    xT = nc.declare_dram_parameter("xT", [KT, 128, NT], bf16, isOutput=False)
    w = nc.declare_dram_parameter("w", [KT, 128, 4 * H], bf16, isOutput=False)
    u = nc.declare_dram_parameter("u", [KT, 128, 4 * H], bf16, isOutput=False)
    bias = nc.declare_dram_parameter("bias", [128, MT], f32, isOutput=False)
    ident = nc.declare_dram_parameter("ident", [128, 128], bf16, isOutput=False)
    y = nc.declare_dram_parameter("y", [128, Tn, KT, BC], bf16, isOutput=True)

    xz = nc.dram_tensor("xz", [4 * H, Tn, BC], bf16)
    xz_m = xz.rearrange("(m p) t b -> p m t b", p=128)
    xz_flat = xz.rearrange("(m p) t b -> p m (t b)", p=128)

    with TileContext(nc) as tc:
        with (
            tc.tile_pool(name="const", bufs=1) as cpool,
            tc.tile_pool(name="state", bufs=1) as spool,
            tc.tile_pool(name="cstate", bufs=1, space="PSUM") as cpsum,
            tc.tile_pool(name="p1psum", bufs=2, space="PSUM") as p1psum,
            tc.tile_pool(name="p1stg", bufs=2) as p1stg,
        ):
            u_sb = [cpool.tile([128, 4 * H], bf16, name=f"u{k}", tag=f"u{k}") for k in range(KT)]
            xT_sb = [cpool.tile([128, NT], bf16, name=f"xT{k}", tag=f"xT{k}") for k in range(KT)]
            w_sb = [cpool.tile([128, 4 * H], bf16, name=f"w{k}", tag=f"w{k}") for k in range(KT)]
            bias_sb = cpool.tile([128, MT], f32, name="bias", tag="bias")
            id_sb = cpool.tile([128, 128], bf16, name="ident", tag="ident")
            nc.sync.dma_start(out=id_sb, in_=ident[:])
            for k in range(KT):
                nc.sync.dma_start(out=u_sb[k], in_=u[k])
                nc.sync.dma_start(out=xT_sb[k], in_=xT[k])
                nc.sync.dma_start(out=w_sb[k], in_=w[k])
            nc.sync.dma_start(out=bias_sb, in_=bias[:])

            h_bf = spool.tile([128, KT, BC], bf16, name="h_bf", tag="h_bf")
            c_st = spool.tile([128, KT, BC], f32, name="c_st", tag="c_st")
            nc.any.memzero(h_bf)
            nc.any.memzero(c_st)

            def p1_mm(ps, m, nci, k):
                return nc.tensor.matmul(
                    ps,
                    lhsT=w_sb[k][:, m * 128 : (m + 1) * 128],
                    rhs=xT_sb[k][:, nci * NCK : (nci + 1) * NCK],
                    start=(k == 0),
                    stop=(k == KT - 1),
                )

            def p1_fin(ps, m, nci):
                stg = p1stg.tile([128, NCK], bf16, name="stg", tag="stg")
                i_stg = nc.scalar.activation(
                    stg, ps, AF.Identity, bias=bias_sb[:, m : m + 1], scale=1.0
                )
                nc.sync.dma_start(
                    out=xz_flat[:, m, nci * NCK : (nci + 1) * NCK], in_=stg
                )
                return i_stg

            # ---- prologue: xz for blocks 0,1 (block tb+1's prefetch-read is
            # emitted at block-tb start, so its writes must precede it in
            # program order -> the queue starts at block 2) ----
            for nci in range(min(2, NBLK)):
                for m in range(MT):
                    ps = p1psum.tile([128, NCK], f32, name="p1ps", tag="p1ps")
                    for k in range(KT):
                        p1_mm(ps, m, nci, k)
                    p1_fin(ps, m, nci)

            # remaining GEMM work, drained 1 item per step (exactly in time:
            # block nci's 64 items complete by the end of block nci-2)
            p1q = [(m, nci, k) for nci in range(2, NBLK) for m in range(MT)
                   for k in range(KT)]
            p1q.reverse()  # consume from the front via pop()

            # ---------------- recurrence with folded-in GEMM ----------------
            with (
                tc.tile_pool(name="rpsum", bufs=1, space="PSUM") as rpsum,
                tc.tile_pool(name="xzblk", bufs=2) as xzpool,
                tc.tile_pool(name="yblk", bufs=2) as ypool,
                tc.tile_pool(name="ztmp", bufs=2) as zpool,
            ):
                xzblk = {}
                xzblk[0] = xzpool.tile([128, MT, TBLK, BC], bf16, name="xzblk", tag="xzblk")
                nc.sync.dma_start(out=xzblk[0], in_=xz_m[:, :, 0:TBLK, :])
                p1ps = None
                for tb in range(NBLK):
                    if tb + 1 < NBLK:
                        xzblk[tb + 1] = xzpool.tile(
                            [128, MT, TBLK, BC], bf16, name="xzblk", tag="xzblk"
                        )
                        nc.sync.dma_start(
                            out=xzblk[tb + 1],
                            in_=xz_m[:, :, (tb + 1) * TBLK : (tb + 2) * TBLK, :],
                        )
                    xzb = xzblk.pop(tb)
                    yblk = ypool.tile([128, TBLK, KT, BC], bf16, name="yblk", tag="yblk")
                    for tr in range(TBLK):
                        ps = {
                            g: rpsum.tile([128, 4, BC], f32, name=f"ps{g}", tag=f"ps{g}")
                            for g in "fcio"
                        }
                        def xzs(g):
                            b = GM[g]
                            return xzb[:, b : b + 4, tr, :]
                        mm_last = {}
                        for g in "fcio":
                            # xz preload: ps[g] = I.T @ xz_slice (sets has_written)
                            nc.tensor.matmul(
                                ps[g], lhsT=id_sb, rhs=xzs(g),
                                start=True, stop=False,
                            )
                            for mi in range(4):
                                m = GM[g] + mi
                                for k in range(KT):
                                    mm_last[g] = nc.tensor.matmul(
                                        ps[g][:, mi, :],
                                        lhsT=u_sb[k][:, m * 128 : (m + 1) * 128],
                                        rhs=h_bf[:, k, :],
                                        start=False,
                                        stop=(mi == 3 and k == KT - 1),
                                    )
                        sg = {g: zpool.tile([128, KT, BC], SDT, name=f"s{g}", tag=f"s{g}")
                              for g in "fio"}
                        # f chain first: retires mid-stream
                        i_sf = nc.vector.tensor_scalar(
                            sg["f"], ps["f"], 1.0, 0.0, AluOpType.min, AluOpType.max
                        )
                        t1 = zpool.tile([128, KT, BC], f32, name="t1", tag="t1")
                        i_t1 = nc.vector.tensor_mul(t1, sg["f"], c_st)
                        # c~ gate: tanh straight off PSUM on ScalarE
                        gt = zpool.tile([128, KT, BC], SDT, name="gt", tag="gt")
                        i_gt = nc.scalar.activation(gt, ps["c"], AF.Tanh)
                        # i gate
                        i_si = nc.vector.tensor_scalar(
                            sg["i"], ps["i"], 1.0, 0.0, AluOpType.min, AluOpType.max
                        )
                        t2 = zpool.tile([128, KT, BC], SDT, name="t2", tag="t2")
                        i_t2 = nc.vector.tensor_mul(t2, sg["i"], gt)
                        i_ca = nc.vector.tensor_add(c_st, t1, t2)
                        th = zpool.tile([128, KT, BC], SDT, name="th", tag="th")
                        i_th = nc.scalar.activation(th, c_st, AF.Tanh)
                        # o tail
                        i_so = nc.vector.tensor_scalar(
                            sg["o"], ps["o"], 1.0, 0.0, AluOpType.min, AluOpType.max
                        )
                        i_h = nc.vector.tensor_mul(h_bf, sg["o"], th)
                        i_y = nc.scalar.copy(
                            yblk[:, tr].rearrange("p k b -> p (k b)"),
                            h_bf.rearrange("p k b -> p (k b)"),
                        )
                        dve_order = [i_sf, i_t1, i_si, i_t2, i_ca, i_so, i_h]
                        for a, b in zip(dve_order, dve_order[1:]):
                            add_dep_helper(b.ins, a.ins, sync=False, reason="dve order")
                        add_dep_helper(i_th.ins, i_gt.ins, sync=False, reason="act order")
                        add_dep_helper(i_y.ins, i_th.ins, sync=False, reason="act order")
                        # folded-in GEMM: drain the work queue after the o-group
                        for _ in range(1):
                            if not p1q:
                                break
                            m, nci, k = p1q.pop()
                            if k == 0:
                                p1ps = p1psum.tile([128, NCK], f32, name="p1ps", tag="p1ps")
                            i_p1 = p1_mm(p1ps, m, nci, k)
                            add_dep_helper(i_p1.ins, mm_last["o"].ins, sync=False, reason="pe order")
                            if k == KT - 1:
                                i_stg = p1_fin(p1ps, m, nci)
                                add_dep_helper(i_stg.ins, i_y.ins, sync=False, reason="act order")
                    nc.sync.dma_start(out=y[:, tb * TBLK : (tb + 1) * TBLK], in_=yblk)
    return nc


def _prep_core_inputs(x, weights, core, Tn=T):
    """weights: dict with all 24 weight arrays (np float32)."""
    d = core // 4
    s = core % 4
    pre = "" if d == 0 else "b"
    gates = ["i", "f", "o", "c"]
    # fold hard_sigmoid affine (0.2, +0.5) into i/f/o weights and biases
    sc = {"i": 0.2, "f": 0.2, "o": 0.2, "c": 1.0}
    sh = {"i": 0.5, "f": 0.5, "o": 0.5, "c": 0.0}
    Wc = np.concatenate([weights[f"W{pre}_{g}"] * sc[g] for g in gates], axis=1)
    Uc = np.concatenate([weights[f"U{pre}_{g}"] * sc[g] for g in gates], axis=1)
    bc = np.concatenate([weights[f"b{pre}_{g}"] * sc[g] + sh[g] for g in gates], axis=0)
    xc = x[s * BC : (s + 1) * BC, :Tn]
    if d == 1:
        xc = xc[:, ::-1]
    # [b, t, d] -> [d, t, b] -> [KT, 128, Tn*BC]
    xTc = np.ascontiguousarray(xc.transpose(2, 1, 0)).reshape(KT, 128, Tn * BC)
    return {
        "xT": xTc.astype(ml_dtypes.bfloat16),
        "w": Wc.reshape(KT, 128, 4 * H).astype(ml_dtypes.bfloat16),
        "u": Uc.reshape(KT, 128, 4 * H).astype(ml_dtypes.bfloat16),
        "bias": np.ascontiguousarray(bc.reshape(MT, 128).T).astype(np.float32),
        "ident": np.eye(128).astype(ml_dtypes.bfloat16),
    }


def _gather(results, Tn=T):
    out = np.empty((B, Tn, H), np.float32)
    for s in range(4):
        acc = None
        for d in range(2):
            yc = np.asarray(results[d * 4 + s]["y"], dtype=np.float32)  # [128, Tn, KT, BC]
            part = yc.transpose(3, 1, 2, 0).reshape(BC, Tn, H)
            acc = part if acc is None else acc + part
        out[s * BC : (s + 1) * BC] = acc
    return out


def run(inputs, Tn=T, trace=False):
    import concourse.bacc as bacc
    from concourse.bass_utils import run_bass_kernel_spmd

    x = np.asarray(inputs["x"], np.float32)
    weights = {k: np.asarray(v, np.float32) for k, v in inputs.items() if k != "x"}
    nc = bacc.Bacc("TRN2", target_bir_lowering=False)
    build(nc, Tn)
    nc.compile()
    in_maps = [_prep_core_inputs(x, weights, c, Tn) for c in range(NCORES)]
    res = run_bass_kernel_spmd(nc, in_maps, list(range(NCORES)), trace=trace)
    return _gather(res.results, Tn), res


def kernel(**inputs):
    out, _ = run(inputs)
    return out


# revision 3
# speedup vs baseline: 1.0006x; 1.0006x over previous
"""BiDirectional LSTM (B=32, T=512, D=H=512, hard_sigmoid gates, output=fwd+bwd sum)
on 8 Trainium2 NeuronCores — v8.

Sharding: core c in 0..7 -> direction d = c//4 (0=fwd, 1=bwd), batch shard s = c%4.
Backward direction realized in data (time-reversed inputs, Theano go_backwards).

v5 (from v4 trace): the tail is DVE-throughput-bound — after the 3rd mm-group's
~370ns completion latency, ~7 serial DVE ops gate the next step. Changes:
  - gate order f, c~, i, o: the f-chain (zf, sf, t1) retires mid-stream; the tail
    window holds only zi, si, t2, ca, zo, so, h.
  - bf16 gate tiles (z/s/gt/t2/h): DVE 2x/4x perf modes (c accumulator stays fp32).
  - c_st lives in PSUM: tanh(c) reads ScalarE's faster PSUM port.
  - phase-1 GEMM fully folded into the stream via a work queue (2 items/step in
    block 0, then 1/step); prologue is just block 0.
  - explicit DVE/ACT/PE ordering via add_dep_helper.
"""

import numpy as np
import ml_dtypes

B, T, D, H = 32, 512, 512, 512
NCORES = 8
BC = B // 4          # 8 samples per core
KT = D // 128        # 4 k-tiles
MT = (4 * H) // 128  # 16 m-tiles; gate col order [i|f|o|c]
GM = {"i": 0, "f": 4, "o": 8, "c": 12}  # m-chunk base per gate


def build(nc, Tn=T):
    import concourse.mybir as mybir
    from concourse.tile import TileContext
    from concourse.alu_op_type import AluOpType
    from concourse.tile_rust import add_dep_helper

    f32 = mybir.dt.float32
    bf16 = mybir.dt.bfloat16
    AF = mybir.ActivationFunctionType
    SDT = bf16  # gate tiles in bf16 (DVE 2x/4x modes); c accumulator stays fp32
    NT = Tn * BC       # GEMM moving free size
    TBLK = 64          # steps per block; one block = one 512-wide GEMM n-chunk
    NCK = TBLK * BC    # 512
    assert Tn % TBLK == 0
    NBLK = Tn // TBLK

    xT = nc.declare_dram_parameter("xT", [KT, 128, NT], bf16, isOutput=False)
    w = nc.declare_dram_parameter("w", [KT, 128, 4 * H], bf16, isOutput=False)
    u = nc.declare_dram_parameter("u", [KT, 128, 4 * H], bf16, isOutput=False)
    bias = nc.declare_dram_parameter("bias", [128, MT], f32, isOutput=False)
    ident = nc.declare_dram_parameter("ident", [128, 128], bf16, isOutput=False)
    y = nc.declare_dram_parameter("y", [128, Tn, KT, BC], bf16, isOutput=True)

    xz = nc.dram_tensor("xz", [4 * H, Tn, BC], bf16)
    xz_m = xz.rearrange("(m p) t b -> p m t b", p=128)
    xz_flat = xz.rearrange("(m p) t b -> p m (t b)", p=128)

    with TileContext(nc) as tc:
        with (
            tc.tile_pool(name="const", bufs=1) as cpool,
            tc.tile_pool(name="state", bufs=1) as spool,
            tc.tile_pool(name="cstate", bufs=1, space="PSUM") as cpsum,
            tc.tile_pool(name="p1psum", bufs=2, space="PSUM") as p1psum,
            tc.tile_pool(name="p1stg", bufs=2) as p1stg,
        ):
            u_sb = [cpool.tile([128, 4 * H], bf16, name=f"u{k}", tag=f"u{k}") for k in range(KT)]
            xT_sb = [cpool.tile([128, NT], bf16, name=f"xT{k}", tag=f"xT{k}") for k in range(KT)]
            w_sb = [cpool.tile([128, 4 * H], bf16, name=f"w{k}", tag=f"w{k}") for k in range(KT)]
            bias_sb = cpool.tile([128, MT], f32, name="bias", tag="bias")
            id_sb = cpool.tile([128, 128], bf16, name="ident", tag="ident")
            nc.sync.dma_start(out=id_sb, in_=ident[:])
            for k in range(KT):
                nc.sync.dma_start(out=u_sb[k], in_=u[k])
                nc.sync.dma_start(out=xT_sb[k], in_=xT[k])
                nc.sync.dma_start(out=w_sb[k], in_=w[k])
            nc.sync.dma_start(out=bias_sb, in_=bias[:])

            h_bf = spool.tile([128, KT, BC], bf16, name="h_bf", tag="h_bf")
            c_st = spool.tile([128, KT, BC], f32, name="c_st", tag="c_st")
            nc.any.memzero(h_bf)
            nc.any.memzero(c_st)

            def p1_mm(ps, m, nci, k):
                return nc.tensor.matmul(
                    ps,
                    lhsT=w_sb[k][:, m * 128 : (m + 1) * 128],
                    rhs=xT_sb[k][:, nci * NCK : (nci + 1) * NCK],
                    start=(k == 0),
                    stop=(k == KT - 1),
                )

            def p1_fin(ps, m, nci):
                stg = p1stg.tile([128, NCK], bf16, name="stg", tag="stg")
                i_stg = nc.scalar.activation(
                    stg, ps, AF.Identity, bias=bias_sb[:, m : m + 1], scale=1.0
                )
                nc.sync.dma_start(
                    out=xz_flat[:, m, nci * NCK : (nci + 1) * NCK], in_=stg
                )
                return i_stg

            # ---- prologue: xz for blocks 0,1 (block tb+1's prefetch-read is
            # emitted at block-tb start, so its writes must precede it in
            # program order -> the queue starts at block 2) ----
            for nci in range(min(2, NBLK)):
                for m in range(MT):
                    ps = p1psum.tile([128, NCK], f32, name="p1ps", tag="p1ps")
                    for k in range(KT):
                        p1_mm(ps, m, nci, k)
                    p1_fin(ps, m, nci)

            # remaining GEMM work, drained 1 item per step (exactly in time:
            # block nci's 64 items complete by the end of block nci-2)
            p1q = [(m, nci, k) for nci in range(2, NBLK) for m in range(MT)
                   for k in range(KT)]
            p1q.reverse()  # consume from the front via pop()

            # ---------------- recurrence with folded-in GEMM ----------------
            with (
                tc.tile_pool(name="rpsum", bufs=1, space="PSUM") as rpsum,
                tc.tile_pool(name="xzblk", bufs=2) as xzpool,
                tc.tile_pool(name="yblk", bufs=2) as ypool,
                tc.tile_pool(name="ztmp", bufs=2) as zpool,
            ):
                gt_abs = zpool.tile([128, 1], mybir.dt.bfloat16, name="gt_abs", tag="gt_abs")
                xzblk = {}
                xzblk[0] = xzpool.tile([128, MT, TBLK, BC], bf16, name="xzblk", tag="xzblk")
                nc.sync.dma_start(out=xzblk[0], in_=xz_m[:, :, 0:TBLK, :])
                p1ps = None
                for tb in range(NBLK):
                    if tb + 1 < NBLK:
                        xzblk[tb + 1] = xzpool.tile(
                            [128, MT, TBLK, BC], bf16, name="xzblk", tag="xzblk"
                        )
                        nc.sync.dma_start(
                            out=xzblk[tb + 1],
                            in_=xz_m[:, :, (tb + 1) * TBLK : (tb + 2) * TBLK, :],
                        )
                    xzb = xzblk.pop(tb)
                    yblk = ypool.tile([128, TBLK, KT, BC], bf16, name="yblk", tag="yblk")
                    for tr in range(TBLK):
                        ps = {
                            g: rpsum.tile([128, 4, BC], f32, name=f"ps{g}", tag=f"ps{g}")
                            for g in "fcio"
                        }
                        def xzs(g):
                            b = GM[g]
                            return xzb[:, b : b + 4, tr, :]
                        mm_last = {}
                        for g in "fcio":
                            # xz preload: ps[g] = I.T @ xz_slice (sets has_written)
                            nc.tensor.matmul(
                                ps[g], lhsT=id_sb, rhs=xzs(g),
                                start=True, stop=False,
                            )
                            for mi in range(4):
                                m = GM[g] + mi
                                for k in range(KT):
                                    mm_last[g] = nc.tensor.matmul(
                                        ps[g][:, mi, :],
                                        lhsT=u_sb[k][:, m * 128 : (m + 1) * 128],
                                        rhs=h_bf[:, k, :],
                                        start=False,
                                        stop=(mi == 3 and k == KT - 1),
                                    )
                        sg = {g: zpool.tile([128, KT, BC], SDT, name=f"s{g}", tag=f"s{g}")
                              for g in "fio"}
                        # f chain first: retires mid-stream
                        i_sf = nc.vector.tensor_scalar(
                            sg["f"], ps["f"], 1.0, 0.0, AluOpType.min, AluOpType.max
                        )
                        t1 = zpool.tile([128, KT, BC], f32, name="t1", tag="t1")
                        i_t1 = nc.vector.tensor_mul(t1, sg["f"], c_st)
                        # c~ gate: tanh straight off PSUM on ScalarE
                        gt = zpool.tile([128, KT, BC], SDT, name="gt", tag="gt")
                        i_gt = nc.scalar.activation(gt, ps["c"], AF.Tanh)
                        # i gate
                        # absorb the ACT tick for gt on DVE while it idles, so
                        # t2 issues without a semaphore check
                        i_abs = nc.vector.tensor_copy(gt_abs, gt[:, 0, 0:1])
                        i_si = nc.vector.tensor_scalar(
                            sg["i"], ps["i"], 1.0, 0.0, AluOpType.min, AluOpType.max
                        )
                        t2 = zpool.tile([128, KT, BC], SDT, name="t2", tag="t2")
                        i_t2 = nc.vector.tensor_mul(t2, sg["i"], gt)
                        i_ca = nc.vector.tensor_add(c_st, t1, t2)
                        th = zpool.tile([128, KT, BC], SDT, name="th", tag="th")
                        i_th = nc.scalar.activation(th, c_st, AF.Tanh)
                        # o tail
                        i_so = nc.vector.tensor_scalar(
                            sg["o"], ps["o"], 1.0, 0.0, AluOpType.min, AluOpType.max
                        )
                        i_h = nc.vector.tensor_mul(h_bf, sg["o"], th)
                        i_y = nc.scalar.copy(
                            yblk[:, tr].rearrange("p k b -> p (k b)"),
                            h_bf.rearrange("p k b -> p (k b)"),
                        )
                        dve_order = [i_sf, i_t1, i_abs, i_si, i_t2, i_ca, i_so, i_h]
                        for a, b in zip(dve_order, dve_order[1:]):
                            add_dep_helper(b.ins, a.ins, sync=False, reason="dve order")
                        add_dep_helper(i_th.ins, i_gt.ins, sync=False, reason="act order")
                        add_dep_helper(i_y.ins, i_th.ins, sync=False, reason="act order")
                        # folded-in GEMM: drain the work queue after the o-group
                        for _ in range(1):
                            if not p1q:
                                break
                            m, nci, k = p1q.pop()
                            if k == 0:
                                p1ps = p1psum.tile([128, NCK], f32, name="p1ps", tag="p1ps")
                            i_p1 = p1_mm(p1ps, m, nci, k)
                            add_dep_helper(i_p1.ins, mm_last["o"].ins, sync=False, reason="pe order")
                            if k == KT - 1:
                                i_stg = p1_fin(p1ps, m, nci)
                                add_dep_helper(i_stg.ins, i_y.ins, sync=False, reason="act order")
                    nc.sync.dma_start(out=y[:, tb * TBLK : (tb + 1) * TBLK], in_=yblk)
    return nc


def _prep_core_inputs(x, weights, core, Tn=T):
    """weights: dict with all 24 weight arrays (np float32)."""
    d = core // 4
    s = core % 4
    pre = "" if d == 0 else "b"
    gates = ["i", "f", "o", "c"]
    # fold hard_sigmoid affine (0.2, +0.5) into i/f/o weights and biases
    sc = {"i": 0.2, "f": 0.2, "o": 0.2, "c": 1.0}
    sh = {"i": 0.5, "f": 0.5, "o": 0.5, "c": 0.0}
    Wc = np.concatenate([weights[f"W{pre}_{g}"] * sc[g] for g in gates], axis=1)
    Uc = np.concatenate([weights[f"U{pre}_{g}"] * sc[g] for g in gates], axis=1)
    bc = np.concatenate([weights[f"b{pre}_{g}"] * sc[g] + sh[g] for g in gates], axis=0)
    xc = x[s * BC : (s + 1) * BC, :Tn]
    if d == 1:
        xc = xc[:, ::-1]
    # [b, t, d] -> [d, t, b] -> [KT, 128, Tn*BC]
    xTc = np.ascontiguousarray(xc.transpose(2, 1, 0)).reshape(KT, 128, Tn * BC)
    return {
        "xT": xTc.astype(ml_dtypes.bfloat16),
        "w": Wc.reshape(KT, 128, 4 * H).astype(ml_dtypes.bfloat16),
        "u": Uc.reshape(KT, 128, 4 * H).astype(ml_dtypes.bfloat16),
        "bias": np.ascontiguousarray(bc.reshape(MT, 128).T).astype(np.float32),
        "ident": np.eye(128).astype(ml_dtypes.bfloat16),
    }


def _gather(results, Tn=T):
    out = np.empty((B, Tn, H), np.float32)
    for s in range(4):
        acc = None
        for d in range(2):
            yc = np.asarray(results[d * 4 + s]["y"], dtype=np.float32)  # [128, Tn, KT, BC]
            part = yc.transpose(3, 1, 2, 0).reshape(BC, Tn, H)
            acc = part if acc is None else acc + part
        out[s * BC : (s + 1) * BC] = acc
    return out


def run(inputs, Tn=T, trace=False):
    import concourse.bacc as bacc
    from concourse.bass_utils import run_bass_kernel_spmd

    x = np.asarray(inputs["x"], np.float32)
    weights = {k: np.asarray(v, np.float32) for k, v in inputs.items() if k != "x"}
    nc = bacc.Bacc("TRN2", target_bir_lowering=False)
    build(nc, Tn)
    nc.compile()
    in_maps = [_prep_core_inputs(x, weights, c, Tn) for c in range(NCORES)]
    res = run_bass_kernel_spmd(nc, in_maps, list(range(NCORES)), trace=trace)
    return _gather(res.results, Tn), res


def kernel(**inputs):
    out, _ = run(inputs)
    return out
